# revision 12
# baseline (speedup 1.0000x reference)
"""DistortionConvLayer Trainium2 kernel (8-core SPMD, Bass/Tile).

Math: the distortion offsets depend only on (h, tap) and are compile-time
constants. Per (h, tap) the bilinear sample rows y0/y1 are fixed rows and the
x-coordinate is w + s with a constant integer shift s and constant fractional
part. Folding the four bilinear corner weights into the conv kernel gives

    out[b,h] = relu( sum_j  G[h,j]^T @ R[h,j]  + bias )            (F x W)

where chunk j has a (row y, shift s) pair,
    R[h,j] = [ Xc[y, w+s] ; Xc[y, w+s+1] ]   (128 x W, c-major, circular x)
    G[h,j] = sum over taps (k, yrow) hitting (y, s):
                [ wy*wx0 * K_k ; wy*wx1 * K_k ]   (128 x F)

G depends only on the runtime conv kernel (a host-side weight repack), so all
G tables are precomputed in numpy and shipped per core; the device program is
pure fp16 matmuls (N=512, two batch images per matmul) accumulating in fp32
PSUM, a fused ReLU+bias on the scalar engine, and DMA.

Sharding (class-aligned): rows 0..127 fall into three slot-pattern classes —
"B" (h=2..42), "A" (h=44..124) and six boundary specials. The SPMD program
must use one slot list per local step t for all cores, so rows are assigned
to (core, t) such that each t-column holds rows of a single class:
  t=0..4   : core p works row 2+5p+t        (all pattern B, union = 10 slots)
  t=5..14  : core p works row 44+10p+(t-5)  (all pattern A, union = 10 slots)
  t=15     : core p works SPECIAL[p], with an optional per-core circular
             x-shift delta folded into its slab cluster (union = 16 slots)
Total 166 slots -> 332 matmuls/core (vs 226/452 for the naive contiguous
blocks whose per-t unions mix A and B patterns).

Each core's slab holds three row clusters (B: 11 rows, A: 16 rows, S: 7 rows)
so the slot row index is pos = t + base(t) + rho uniformly across cores.
"""

import numpy as np

# problem dims (hardcoded per spec)
B, H, W, C, F = 4, 128, 256, 64, 128
KH = KW = 3
IN_H, IN_W = H + 2, W + 2
NCORE = 8
NH = H // NCORE            # h rows per core
ROWQ = 260                 # stored row width: q in [0,260) holds circ col (q-1-d)
MARG = 1                   # read offset margin: slot sigma >= -1
NROW = 34                  # slab rows: cluster B 11 + cluster A 16 + cluster S 7
SPECIAL = (0, 1, 42, 43, 124, 125, 126, 127)
SDELTA = (0, 0, 0, 0, -1, 0, 0, 0)   # per-core x-shift for the specials column
BASE = tuple(2 if t < 5 else (8 if t < 15 else 14) for t in range(NH))


# ---------------------------------------------------------------- host tables
def _make_offset(h, w, dilation=1.0, skydome=True):
    pi = np.pi
    unit_w = 2.0 * pi / w
    unit_h = pi / (2.0 * h) if skydome else pi / h
    rho = np.tan(unit_w) * dilation
    v = np.array([0.0, 1.0, 0.0])
    r_grid = np.array(
        [[1, -1], [1, 0], [1, 1], [0, -1], [0, 0], [0, 1], [-1, -1], [-1, 0], [-1, 1]],
        dtype=np.float64,
    )
    xc = int(w * 0.5)
    theta = (xc - 0.5 * w) * unit_w
    y = np.arange(h, dtype=np.float64)
    phi = (h - y) * unit_h if skydome else (h * 0.5 - y) * unit_h
    p_u = np.stack(
        [np.cos(phi) * np.cos(theta), np.sin(phi), np.cos(phi) * np.sin(theta)], axis=-1
    )
    t_x = np.cross(np.broadcast_to(v, p_u.shape), p_u)
    t_y = np.cross(p_u, t_x)
    r_sphere = rho * (
        r_grid[None, :, 0, None] * t_x[:, None, :]
        + r_grid[None, :, 1, None] * t_y[:, None, :]
    )
    p_ur = p_u[:, None, :] + r_sphere
    ux, uy, uz = p_ur[..., 0], p_ur[..., 1], p_ur[..., 2]
    base = np.arctan2(uz, ux)
    theta_r = np.where(
        ux > 0,
        base,
        np.where(
            ux < 0,
            np.where(uz >= 0, base + pi, base - pi),
            np.where(uz > 0, pi * 0.5, -pi * 0.5),
        ),
    )
    phi_r = np.arcsin(uy)
    x_r = (theta_r / pi + 1.0) * 0.5 * w
    y_r = (1.0 - 2.0 * phi_r / pi) * h if skydome else (0.5 - phi_r / pi) * h
    k = np.stack([x_r, y_r], axis=-1)
    off = k - k[:, 4:5, :]
    return off.astype(np.float32)  # [h, 9, 2]


def _build_chunk_tables():
    """Per-h chunk decomposition.

    Returns (chunks, terms): chunks[h] = [(y, s)], terms[h] = list of
    (tap k, chunk idx, a_top, a_bot) with 18 entries.
    """
    off = _make_offset(H, W)
    chunks_all, terms_all = [], []
    for h in range(H):
        ids, chunks, terms = {}, [], []
        for k in range(KH * KW):
            dy, dx = k // 3, k % 3
            cy, cx = np.float32(off[h, k, 0]), np.float32(off[h, k, 1])
            yv = float(np.float32(h + dy) + cy)
            yv = min(max(yv, 0.0), float(IN_H - 1))
            y0 = min(max(int(np.floor(yv)), 0), IN_H - 1)
            y1 = min(y0 + 1, IN_H - 1)
            wy0, wy1 = float(y1 - yv), float(yv - y0)
            s = dx + int(np.floor(cx))
            fx = float(dx + cx - np.floor(cx + dx))
            wx0, wx1 = 1.0 - fx, fx
            for yy, wy in ((y0, wy0), (y1, wy1)):
                if wy == 0.0:
                    continue
                key = (yy, s)
                if key not in ids:
                    ids[key] = len(chunks)
                    chunks.append(key)
                terms.append((k, ids[key], wy * wx0, wy * wx1))
        chunks_all.append(chunks)
        terms_all.append(terms)
    return chunks_all, terms_all


def _corner_sets(chunks_all, terms_all):
    """Per h: list of (rho, sigma, weight, tap) corner contributions."""
    corners_all = []
    for h in range(H):
        chunks, terms = chunks_all[h], terms_all[h]
        cs = []
        for (k, j, a_top, a_bot) in terms:
            y, sg = chunks[j]
            if a_top != 0.0:
                cs.append((y - h, sg, a_top, k))
            if a_bot != 0.0:
                cs.append((y - h, sg + 1, a_bot, k))
        corners_all.append(cs)
    return corners_all


def _row_of():
    """(core, t) -> image row h."""
    row = np.zeros((NCORE, NH), np.int64)
    for p in range(NCORE):
        for t in range(5):
            row[p, t] = 2 + 5 * p + t
        for t in range(5, 15):
            row[p, t] = 44 + 10 * p + (t - 5)
        row[p, 15] = SPECIAL[p]
    return row


def _greedy_cover(cells):
    need = set(cells)
    slots, needc = [], set(need)
    while needc:
        best, bc = None, -1
        for (r, sg) in sorted(needc):
            for cand in ((r, sg), (r, sg - 1)):
                cov = len({(cand[0], cand[1]), (cand[0], cand[1] + 1)} & needc)
                if cov > bc:
                    bc, best = cov, cand
        slots.append(best)
        needc -= {(best[0], best[1]), (best[0], best[1] + 1)}
    return sorted(slots)


def _build_static_plan(corners_all):
    """Per-t slot lists: union over cores of the (shifted) corner cells."""
    row = _row_of()
    slots_all = []
    for t in range(NH):
        cells = set()
        for p in range(NCORE):
            d = SDELTA[p] if t == 15 else 0
            cells |= {(r, sg + d) for (r, sg, _w, _k) in corners_all[row[p, t]]}
        slots = _greedy_cover(cells)
        for (r, sg) in slots:
            assert -1 <= sg <= 3
        for (r, sg) in cells:
            assert -1 <= sg <= 3
            assert (r, sg) in slots or (r, sg - 1) in slots
        slots_all.append(slots)
    return row, slots_all


def _core_g_tables(core, corners_all, row_of, slots_all, kernel):
    """Host-computed per-core G tables [128, sum_t nslot(t)*128] fp16.
    Each corner contribution is assigned to one covering slot (top half if
    slot s == sigma, else bottom half of slot s == sigma-1)."""
    totg = sum(len(sl) for sl in slots_all)
    g = np.zeros((128, totg * 128), np.float32)
    goff = 0
    for t in range(NH):
        slots = slots_all[t]
        sid = {key: i for i, key in enumerate(slots)}
        d = SDELTA[core] if t == 15 else 0
        for (r, sg, w, k) in corners_all[row_of[core, t]]:
            sg = sg + d
            Kk = kernel[k * C : (k + 1) * C, :]
            if (r, sg) in sid:
                i, half = sid[(r, sg)], 0
            else:
                i, half = sid[(r, sg - 1)], 1
            lo = 64 * half
            g[lo : lo + 64, (goff + i) * 128 : (goff + i + 1) * 128] += np.float32(w) * Kk
        goff += len(slots)
    return np.ascontiguousarray(g.astype(np.float16))


def _core_input_slab(xpc, core):
    """xpc: [B, C, IN_H, IN_W] padded channel-major input.
    Returns [B, C, NROW, ROWQ] f32 slab: three clusters of padded-image rows
    with circular x layout (col q holds circ col (q-1-d) mod IN_W)."""
    hs = SPECIAL[core]
    spans = [
        (5 * core, 11, 0),                 # cluster B: pos 0..10
        (42 + 10 * core, 16, 0),           # cluster A: pos 11..26
        (hs - 2, 7, SDELTA[core]),         # cluster S: pos 27..33
    ]
    slab = np.zeros((B, C, NROW, ROWQ), np.float32)
    pos = 0
    for (y0, n, d) in spans:
        ys = np.arange(y0, y0 + n)
        valid = (ys >= 0) & (ys < IN_H)
        rows = np.zeros((B, C, n, IN_W), np.float32)
        rows[:, :, valid, :] = xpc[:, :, ys[valid], :]
        cols = (np.arange(ROWQ) - 1 - d) % IN_W
        slab[:, :, pos : pos + n, :] = rows[:, :, :, cols]
        pos += n
    assert pos == NROW
    return np.ascontiguousarray(slab)


# ---------------------------------------------------------------- device code
def build_program():
    """Uniform SPMD Bass program: pure matmul + relu (G precomputed on host)."""
    import concourse.mybir as mybir
    import concourse.tile as tile
    from concourse import bacc
    from concourse.bass import ts

    f32 = mybir.dt.float32
    f16 = mybir.dt.float16

    chunks_all, terms_all = _build_chunk_tables()
    corners_all = _corner_sets(chunks_all, terms_all)
    row_of, slots_all = _build_static_plan(corners_all)
    totg = sum(len(sl) for sl in slots_all)

    nc = bacc.Bacc("TRN2", target_bir_lowering=False, debug=False)

    xs_d = nc.dram_tensor("xs", [B, C, NROW, ROWQ], f16, kind="ExternalInput").ap()
    g_d = nc.dram_tensor("g", [128, totg * 128], f16, kind="ExternalInput").ap()
    bias_d = nc.dram_tensor("bias", [F], f32, kind="ExternalInput").ap()
    out_d = nc.dram_tensor("out", [NH, F, B, W], f16, kind="ExternalOutput").ap()

    with tile.TileContext(nc) as tc:
        with (
            tc.tile_pool(name="const", bufs=1) as cpool,
            tc.tile_pool(name="pspool", bufs=4, space="PSUM") as pspool,
            tc.tile_pool(name="stpool", bufs=4) as stpool,
        ):
            xst = cpool.tile([128, B, NROW * ROWQ], f16)
            gtile = cpool.tile([128, totg * 128], f16)
            btile = cpool.tile([128, 1], f32)
            src_top = xs_d.rearrange("b c r q -> c b (r q)")
            flat_n = NROW * ROWQ

            # The stream is DMA-bound: backend descriptor throughput caps the
            # effective rate, and dependent DMAs block their ring FIFO. So:
            # sync ring = X top halves, scalar ring = G, gpsimd ring = the
            # +1-shifted bottom halves (first rows straight from HBM to avoid
            # a dependency chain at startup, the rest as SBUF->SBUF copies).
            g_bounds = [0]
            for sl in slots_all:
                g_bounds.append(g_bounds[-1] + len(sl) * 128)
            gb = [b // 128 for b in g_bounds]

            def emit_g(c0, c1):
                nc.scalar.dma_start(
                    gtile[:, c0 * 128 : c1 * 128], g_d[:, c0 * 128 : c1 * 128]
                )

            def emit_top(r0, r1):
                c0, c1 = r0 * ROWQ, r1 * ROWQ
                nc.sync.dma_start(xst[0:64, :, c0:c1], src_top[:, :, c0:c1])

            def emit_bot_hbm(r0, r1):
                c0, c1 = r0 * ROWQ, min(r1 * ROWQ + 1, flat_n)
                nc.gpsimd.dma_start(
                    xst[64:128, :, c0 : c1 - 1], src_top[:, :, c0 + 1 : c1]
                )

            def emit_bot_copy(r0, r1):
                # bottom[q] = top[q+1]; reads one element past c1, so the top
                # chunks must cover [r0*Q+1, r1*Q+1) (tile tracker orders it).
                c0, c1 = r0 * ROWQ, min(r1 * ROWQ + 1, flat_n)
                nc.gpsimd.dma_start(
                    xst[64:128, :, c0 : c1 - 1], xst[0:64, :, c0 + 1 : c1]
                )

            emit_g(0, 1)
            emit_top(0, 1)
            emit_bot_hbm(0, 2)
            emit_g(1, 4)
            nc.scalar.dma_start(btile[:, :], bias_d.rearrange("f -> f ()"))
            emit_top(1, 2)
            emit_bot_hbm(2, 4)
            emit_top(2, 4)
            emit_bot_hbm(4, 6)
            emit_g(4, gb[1])
            emit_top(4, 6)
            emit_bot_hbm(6, 8)
            emit_top(6, 9)
            emit_g(gb[1], gb[2])
            emit_top(9, 12)
            emit_g(gb[2], gb[3])
            emit_bot_copy(8, 11)
            emit_g(gb[3], gb[4])
            emit_top(12, 16)
            emit_bot_copy(11, 14)
            emit_g(gb[4], gb[5])
            emit_top(16, 20)
            emit_bot_copy(14, 17)
            emit_g(gb[5], gb[7])
            emit_bot_copy(17, 19)
            emit_top(20, 24)
            emit_g(gb[7], gb[9])
            emit_bot_copy(19, 22)
            emit_top(24, 28)
            emit_g(gb[9], gb[11])
            emit_bot_copy(22, 25)
            emit_top(28, 34)
            emit_g(gb[11], gb[13])
            emit_bot_copy(25, 28)
            emit_g(gb[13], gb[15])
            emit_bot_copy(28, 31)
            emit_g(gb[15], gb[16])
            emit_bot_copy(31, 34)
            # bottom half's final flat element is never covered by the shifted
            # copies; write something finite so 0-weight G rows can't see NaN.
            nc.scalar.dma_start(
                xst[64:128, :, flat_n - 1 : flat_n], src_top[:, :, flat_n - 1 : flat_n]
            )

            relu = mybir.ActivationFunctionType.Relu

            goff = 0
            for t in range(NH):
                slots = slots_all[t]
                nslot = len(slots)
                ps0 = pspool.tile([128, 2, 256], f32)
                ps1 = pspool.tile([128, 2, 256], f32)
                for bp, pst in ((0, ps0), (1, ps1)):
                    for j, (rho, sig) in enumerate(slots):
                        off = (t + BASE[t] + rho) * ROWQ + (sig + MARG)
                        nc.tensor.matmul(
                            pst[:, :, :],
                            lhsT=gtile[:, ts(goff + j, 128)],
                            rhs=xst[:, 2 * bp : 2 * bp + 2, off : off + 256],
                            start=(j == 0),
                            stop=(j == nslot - 1),
                        )
                goff += nslot
                st = stpool.tile([128, B, 256], f16)
                out_eng = [nc.sync, nc.scalar]
                nc.scalar.activation(st[:, 0:2, :], ps0[:, :, :], relu, bias=btile[:, 0:1])
                out_eng[t % 2].dma_start(out_d[t][:, 0:2], st[:, 0:2, :])
                nc.scalar.activation(st[:, 2:4, :], ps1[:, :, :], relu, bias=btile[:, 0:1])
                out_eng[(t + 1) % 2].dma_start(out_d[t][:, 2:4], st[:, 2:4, :])
            assert goff == totg

    nc.compile()
    return nc


def make_in_maps(inputs, kernel, bias):
    chunks_all, terms_all = _build_chunk_tables()
    corners_all = _corner_sets(chunks_all, terms_all)
    row_of, slots_all = _build_static_plan(corners_all)
    xp = np.pad(inputs.astype(np.float32), ((0, 0), (1, 1), (1, 1), (0, 0)))
    xpc = np.ascontiguousarray(xp.transpose(0, 3, 1, 2))  # [B, C, IN_H, IN_W]
    kf = np.asarray(kernel, np.float32)
    bs = np.ascontiguousarray(bias.astype(np.float32))
    in_maps = []
    for core in range(NCORE):
        in_maps.append(
            {
                "xs": _core_input_slab(xpc, core).astype(np.float16),
                "g": _core_g_tables(core, corners_all, row_of, slots_all, kf),
                "bias": bs,
            }
        )
    return in_maps


_PROGRAM_CACHE = {}


def kernel(inputs, kernel, bias):
    from concourse import bass_utils

    if "nc" not in _PROGRAM_CACHE:
        _PROGRAM_CACHE["nc"] = build_program()
    nc = _PROGRAM_CACHE["nc"]
    in_maps = make_in_maps(np.asarray(inputs), np.asarray(kernel), np.asarray(bias))
    res = bass_utils.run_bass_kernel_spmd(nc, in_maps, core_ids=list(range(NCORE)))
    row_of = _row_of()
    out = np.empty((B, H, W, F), np.float32)
    for core in range(NCORE):
        o = np.asarray(res.results[core]["out"], np.float32)  # [NH, F, B, W]
        for t in range(NH):
            out[:, row_of[core, t]] = o[t].transpose(1, 2, 0)
    return out
